# revision 24
# baseline (speedup 1.0000x reference)
"""DenseMatcher kernel for 8 TRN2 NeuronCores.

sim = (q/|q|)^T (p/|p|)  [9216, 9216], row-argmax, col-argmax, mutual-NN.

Sharding: core k owns sim rows [k*1152, (k+1)*1152) (GEMM1: q_shard^T @ p_full,
fp32r) and sim cols [k*1152, (k+1)*1152) (GEMM2: p_shard^T @ q_full, bf16,
whose row-argmax is the col-argmax of sim). Device does normalize, both GEMMs
and full top-8 argmax scans per half-row; the host concatenates per-core
results and rescores the (<=16) candidates per row/col at f64 to undo the
reduced-precision GEMM noise (noise ~1e-4..3e-3 << top-2 gap median 1.3e-2,
so the exact argmax is always inside the device candidate set).
"""

import numpy as np
import sys

sys.path.insert(0, "/opt/trn_rl_repo")

import concourse.bacc as bacc
import concourse.bass as bass
import concourse.mybir as mybir
from concourse.tile import TileContext

F32 = mybir.dt.float32
F32R = mybir.dt.float32r
BF16 = mybir.dt.bfloat16
U32 = mybir.dt.uint32

C = 256  # channels (2 partition tiles of 128)
H = 96
W = 96
NTOT = H * W  # 9216
NCORES = 8
NS = NTOT // NCORES  # 1152 rows/cols per core
G = NS // 128  # 9 groups of 128 rows
CHUNK = 512
THRESHOLD = 0.9

ACTF = mybir.ActivationFunctionType


def build_graph(ntot=NTOT, ns=NS, do_compile=True):
    g = ns // 128
    half = ntot // 2
    nch_half = half // CHUNK
    assert half % CHUNK == 0

    nc = bacc.Bacc("TRN2", target_bir_lowering=False, debug=False)
    q_full = nc.declare_dram_parameter("q_full", [C, ntot], F32, isOutput=False)
    p_full = nc.declare_dram_parameter("p_full", [C, ntot], F32, isOutput=False)
    q_shard = nc.declare_dram_parameter("q_shard", [C, ns], F32, isOutput=False)
    p_shard = nc.declare_dram_parameter("p_shard", [C, ns], F32, isOutput=False)

    sim_out = nc.declare_dram_parameter("sim", [ns, ntot], F32, isOutput=True)
    row_vals_o = nc.declare_dram_parameter("row_vals", [128, g * 16], F32, isOutput=True)
    row_idx_o = nc.declare_dram_parameter("row_idx", [128, g * 16], U32, isOutput=True)
    col_vals_o = nc.declare_dram_parameter("col_vals", [128, g * 16], F32, isOutput=True)
    col_idx_o = nc.declare_dram_parameter("col_idx", [128, g * 16], U32, isOutput=True)

    def rd(ap):
        # f32-view for non-matmul readers of fp32r tiles; bf16 reads natively
        return ap.bitcast(F32) if ap.dtype == F32R else ap

    with TileContext(nc) as tc:
        with (
            tc.tile_pool(name="pstr", bufs=1) as pool_pstr,
            tc.tile_pool(name="qstr", bufs=1) as pool_qstr,
            tc.tile_pool(name="block", bufs=2) as pool_block,
            tc.tile_pool(name="shard", bufs=1) as pool_shard,
            tc.tile_pool(name="sq", bufs=2) as pool_sq,
            tc.tile_pool(name="res", bufs=1) as pool_res,
            tc.tile_pool(name="psum", bufs=4, space="PSUM") as pool_psum,
            tc.tile_pool(name="const", bufs=1) as pool_const,
        ):
            ones = pool_const.tile([128, 128], F32, tag="ones")
            nc.vector.memset(ones, 1.0)

            row_vals = pool_res.tile([128, g * 16], F32, tag="rv")
            row_idx = pool_res.tile([128, g * 16], U32, tag="ri")
            col_vals = pool_res.tile([128, g * 16], F32, tag="cv")
            col_idx = pool_res.tile([128, g * 16], U32, tag="ci")

            def load_pair(pool, dram, width, tag, dt):
                # gpsimd casting DMA rounds f32 -> f32r/bf16 on load, so every
                # writer of a matmul input is reduced-precision-rounded.
                t0 = pool.tile([128, width], dt, tag=tag + "0")
                t1 = pool.tile([128, width], dt, tag=tag + "1")
                nc.gpsimd.dma_start(out=t0, in_=dram[0:128, :])
                nc.gpsimd.dma_start(out=t1, in_=dram[128:256, :])
                return t0, t1

            def normalize(t0, t1, width):
                """Column-L2-normalize the c-tile pair in place, fully chunked:
                squares (ACT) -> colsum broadcast (PE ones-matmul) -> sqrt (ACT)
                -> approx-reciprocal (DVE) -> scale (Pool). inv chunks live only
                transiently in the sq pool."""
                for ck in range(0, width, CHUNK):
                    w = min(CHUNK, width - ck)
                    ts0 = pool_sq.tile([128, CHUNK], F32, tag="ts0")
                    ts1 = pool_sq.tile([128, CHUNK], F32, tag="ts1")
                    nc.scalar.activation(ts0[:, :w], rd(t0[:, ck : ck + w]), ACTF.Square)
                    nc.scalar.activation(ts1[:, :w], rd(t1[:, ck : ck + w]), ACTF.Square)
                    ps = pool_psum.tile([128, 2 * CHUNK], F32, tag="ps")
                    nc.tensor.matmul(ps[:, :w], ones, ts0[:, :w], start=True, stop=False)
                    nc.tensor.matmul(ps[:, :w], ones, ts1[:, :w], start=False, stop=True)
                    inv = pool_sq.tile([128, CHUNK], F32, tag="inv")
                    scr = pool_sq.tile([128, CHUNK], F32, tag="scr")
                    nc.scalar.activation(inv[:, :w], ps[:, :w], ACTF.Sqrt)
                    nc.vector.reciprocal_approx_accurate(
                        inv[:, :w], inv[:, :w], scr[:, :w]
                    )
                    nc.gpsimd.tensor_tensor(
                        t0[:, ck : ck + w], rd(t0[:, ck : ck + w]), inv[:, :w],
                        mybir.AluOpType.mult,
                    )
                    nc.gpsimd.tensor_tensor(
                        t1[:, ck : ck + w], rd(t1[:, ck : ck + w]), inv[:, :w],
                        mybir.AluOpType.mult,
                    )

            # ---- loads all up front (q/ps stream in during GEMM1) ----
            p0, p1 = load_pair(pool_pstr, p_full, ntot, "strp", F32R)
            q0, q1 = load_pair(pool_qstr, q_full, ntot, "strq", BF16)
            qs0, qs1 = load_pair(pool_shard, q_shard, ns, "qs", F32R)
            ps0, ps1 = load_pair(pool_shard, p_shard, ns, "psh", BF16)

            normalize(p0, p1, ntot)
            normalize(qs0, qs1, ns)

            def gemm_scan(w0, w1, s0, s1, vals, idxs, dma_sim):
                """w: [128, ns] weight c-tiles; s: [128, ntot] stream c-tiles.
                Per 128-row group: two half-row blocks, each evicted to SBUF,
                scanned with max8 + find_index8 (8 candidates per half)."""
                for gi in range(g):
                    lhs0 = w0[:, gi * 128 : (gi + 1) * 128]
                    lhs1 = w1[:, gi * 128 : (gi + 1) * 128]
                    for h in range(2):
                        blk = pool_block.tile([128, half], F32, tag="blk")
                        base = h * nch_half
                        cb = 0
                        while cb < nch_half:
                            npair = min(2, nch_half - cb)
                            wdt = npair * CHUNK
                            ps = pool_psum.tile([128, 2 * CHUNK], F32, tag="ps")
                            for lhs, s, st in ((lhs0, s0, True), (lhs1, s1, False)):
                                for i in range(npair):
                                    ck = base + cb + i
                                    nc.tensor.matmul(
                                        ps[:, i * CHUNK : (i + 1) * CHUNK],
                                        lhs,
                                        s[:, ck * CHUNK : (ck + 1) * CHUNK],
                                        start=st, stop=not st,
                                    )
                            nc.scalar.activation(
                                blk[:, cb * CHUNK : cb * CHUNK + wdt],
                                ps[:, :wdt], ACTF.Copy,
                            )
                            cb += npair
                        if dma_sim is not None:
                            nc.sync.dma_start(
                                out=dma_sim[
                                    gi * 128 : (gi + 1) * 128,
                                    h * half : (h + 1) * half,
                                ],
                                in_=blk,
                            )
                        sl = slice(gi * 16 + h * 8, gi * 16 + h * 8 + 8)
                        nc.vector.max(out=vals[:, sl], in_=blk)
                        nc.vector.max_index(
                            out=idxs[:, sl], in_max=vals[:, sl], in_values=blk
                        )

            # ---- GEMM1 (fp32r): sim rows + row-argmax candidates ----
            gemm_scan(qs0, qs1, p0, p1, row_vals, row_idx, sim_out)
            nc.sync.dma_start(out=row_vals_o[:], in_=row_vals)
            nc.sync.dma_start(out=row_idx_o[:], in_=row_idx)

            # ---- q/p_shard norms run in GEMM1's engine tail ----
            normalize(q0, q1, ntot)
            normalize(ps0, ps1, ns)

            # ---- GEMM2 (bf16): col-argmax candidates, no sim writeback ----
            gemm_scan(ps0, ps1, q0, q1, col_vals, col_idx, None)
            nc.sync.dma_start(out=col_vals_o[:], in_=col_vals)
            nc.sync.dma_start(out=col_idx_o[:], in_=col_idx)

    if do_compile:
        nc.compile()
    return nc


_CACHED = {}


def _get_graph():
    if "nc" not in _CACHED:
        _CACHED["nc"] = build_graph()
    return _CACHED["nc"]


def _unpack_cand(arr, g=G, half=NTOT // 2):
    """[128, g*16] -> [g*128, 16] global candidate indices in shard row order.
    Slots 8:16 come from the second half-row block -> +half offset."""
    a = arr.reshape(128, g, 2, 8).astype(np.int64)
    a = a.copy()
    a[:, :, 1, :] += half
    return a.transpose(1, 0, 2, 3).reshape(g * 128, 16)


def _refine_argmax(cand_idx, qn, pn, transpose):
    """Exact f64 rescoring of the device's candidates per row.

    cand_idx: [R, K] candidate indices. For rows (transpose=False) row r of sim
    scores candidates as qn[:, r] . pn[:, cand]; for cols the roles swap.
    Returns (argmax_index [R], max_value [R])."""
    a, b = (qn, pn) if not transpose else (pn, qn)
    scores = np.einsum("cr,crk->rk", a, b[:, cand_idx], optimize=True)
    best = scores.max(axis=1, keepdims=True)
    # break exact ties by smallest index (jnp argmax semantics)
    masked = np.where(scores == best, cand_idx, np.iinfo(np.int64).max)
    idx = masked.min(axis=1)
    return idx, best[:, 0]


def kernel(feature_query: np.ndarray, feature_projection: np.ndarray):
    from concourse.bass_utils import run_bass_kernel_spmd

    q = np.ascontiguousarray(np.asarray(feature_query, np.float32).reshape(C, NTOT))
    p = np.ascontiguousarray(np.asarray(feature_projection, np.float32).reshape(C, NTOT))

    in_maps = []
    for k in range(NCORES):
        sl = slice(k * NS, (k + 1) * NS)
        in_maps.append(
            {
                "q_full": q,
                "p_full": p,
                "q_shard": np.ascontiguousarray(q[:, sl]),
                "p_shard": np.ascontiguousarray(p[:, sl]),
            }
        )

    nc = _get_graph()
    res = run_bass_kernel_spmd(nc, in_maps, core_ids=list(range(NCORES))).results

    sim = np.concatenate([res[k]["sim"] for k in range(NCORES)], axis=0)
    row_cand = np.concatenate([_unpack_cand(res[k]["row_idx"]) for k in range(NCORES)])
    col_cand = np.concatenate([_unpack_cand(res[k]["col_idx"]) for k in range(NCORES)])

    q64 = q.astype(np.float64)
    p64 = p.astype(np.float64)
    qn = q64 / np.linalg.norm(q64, axis=0, keepdims=True)
    pn = p64 / np.linalg.norm(p64, axis=0, keepdims=True)
    row_max, _ = _refine_argmax(row_cand, qn, pn, transpose=False)
    q_idx, sim_vals = _refine_argmax(col_cand, qn, pn, transpose=True)

    j = np.arange(NTOT)
    mutual = (row_max[q_idx] == j) & (sim_vals > THRESHOLD)
    q_div, q_mod = q_idx // W, q_idx % W
    p_div, p_mod = j // W, j % W
    valid = mutual & (q_div + 1 < H) & (p_div + 1 < H)

    i32 = np.int32
    return (
        sim,
        valid,
        q_idx.astype(i32),
        q_div.astype(i32),
        q_mod.astype(i32),
        p_div.astype(i32),
        p_mod.astype(i32),
    )


# revision 27
# speedup vs baseline: 1.2158x; 1.2158x over previous
"""DenseMatcher kernel for 8 TRN2 NeuronCores.

sim = (q/|q|)^T (p/|p|)  [9216, 9216], row-argmax, col-argmax, mutual-NN.

Sharding: core k owns sim rows [k*1152, (k+1)*1152) (GEMM1: q_shard^T @ p_full,
fp32r) and sim cols [k*1152, (k+1)*1152) (GEMM2: p_shard^T @ q_full, bf16,
whose row-argmax is the col-argmax of sim). Device does normalize, both GEMMs
and full top-8 argmax scans per half-row; the host concatenates per-core
results and rescores the (<=16) candidates per row/col at f64 to undo the
reduced-precision GEMM noise (noise ~1e-4..3e-3 << top-2 gap median 1.3e-2,
so the exact argmax is always inside the device candidate set).
"""

import numpy as np
import sys

sys.path.insert(0, "/opt/trn_rl_repo")

import concourse.bacc as bacc
import concourse.bass as bass
import concourse.mybir as mybir
from concourse.tile import TileContext

F32 = mybir.dt.float32
F32R = mybir.dt.float32r
BF16 = mybir.dt.bfloat16
U32 = mybir.dt.uint32

C = 256  # channels (2 partition tiles of 128)
H = 96
W = 96
NTOT = H * W  # 9216
NCORES = 8
NS = NTOT // NCORES  # 1152 rows/cols per core
G = NS // 128  # 9 groups of 128 rows
CHUNK = 512
THRESHOLD = 0.9

ACTF = mybir.ActivationFunctionType


def build_graph(ntot=NTOT, ns=NS, do_compile=True):
    g = ns // 128
    half = ntot // 2
    nch_half = half // CHUNK
    assert half % CHUNK == 0

    nc = bacc.Bacc("TRN2", target_bir_lowering=False, debug=False)
    q_full = nc.declare_dram_parameter("q_full", [C, ntot], F32, isOutput=False)
    p_full = nc.declare_dram_parameter("p_full", [C, ntot], F32, isOutput=False)
    q_shard = nc.declare_dram_parameter("q_shard", [C, ns], F32, isOutput=False)
    p_shard = nc.declare_dram_parameter("p_shard", [C, ns], F32, isOutput=False)

    sim_out = nc.declare_dram_parameter("sim", [ns, ntot], F32, isOutput=True)
    row_vals_o = nc.declare_dram_parameter("row_vals", [128, g * 16], F32, isOutput=True)
    row_idx_o = nc.declare_dram_parameter("row_idx", [128, g * 16], U32, isOutput=True)
    col_vals_o = nc.declare_dram_parameter("col_vals", [128, g * 16], F32, isOutput=True)
    col_idx_o = nc.declare_dram_parameter("col_idx", [128, g * 16], U32, isOutput=True)

    def rd(ap):
        # f32-view for non-matmul readers of fp32r tiles; bf16 reads natively
        return ap.bitcast(F32) if ap.dtype == F32R else ap

    with TileContext(nc) as tc:
        with (
            tc.tile_pool(name="pstr", bufs=1) as pool_pstr,
            tc.tile_pool(name="qstr", bufs=1) as pool_qstr,
            tc.tile_pool(name="block", bufs=2) as pool_block,
            tc.tile_pool(name="shard", bufs=1) as pool_shard,
            tc.tile_pool(name="sq", bufs=2) as pool_sq,
            tc.tile_pool(name="res", bufs=1) as pool_res,
            tc.tile_pool(name="psum", bufs=4, space="PSUM") as pool_psum,
            tc.tile_pool(name="const", bufs=1) as pool_const,
        ):
            ones = pool_const.tile([128, 128], F32, tag="ones")
            nc.vector.memset(ones, 1.0)

            row_vals = pool_res.tile([128, g * 16], F32, tag="rv")
            row_idx = pool_res.tile([128, g * 16], U32, tag="ri")
            col_vals = pool_res.tile([128, g * 16], F32, tag="cv")
            col_idx = pool_res.tile([128, g * 16], U32, tag="ci")

            def load_pair(pool, dram, width, tag, dt, pieces=1):
                # gpsimd casting DMA rounds f32 -> f32r/bf16 on load, so every
                # writer of a matmul input is reduced-precision-rounded.
                # Chunked pieces let the norm pipeline start at the first one.
                t0 = pool.tile([128, width], dt, tag=tag + "0")
                t1 = pool.tile([128, width], dt, tag=tag + "1")
                step = width // pieces
                for o in range(0, width, step):
                    e = min(o + step, width)
                    nc.gpsimd.dma_start(out=t0[:, o:e], in_=dram[0:128, o:e])
                    nc.gpsimd.dma_start(out=t1[:, o:e], in_=dram[128:256, o:e])
                return t0, t1

            def normalize(t0, t1, width):
                """Column-L2-normalize the c-tile pair in place, fully chunked:
                squares (ACT) -> colsum broadcast (PE ones-matmul) -> sqrt (ACT)
                -> approx-reciprocal (DVE) -> scale (Pool). inv chunks live only
                transiently in the sq pool."""
                for ck in range(0, width, CHUNK):
                    w = min(CHUNK, width - ck)
                    ts0 = pool_sq.tile([128, CHUNK], F32, tag="ts0")
                    ts1 = pool_sq.tile([128, CHUNK], F32, tag="ts1")
                    nc.scalar.activation(ts0[:, :w], rd(t0[:, ck : ck + w]), ACTF.Square)
                    nc.scalar.activation(ts1[:, :w], rd(t1[:, ck : ck + w]), ACTF.Square)
                    ps = pool_psum.tile([128, 2 * CHUNK], F32, tag="ps")
                    nc.tensor.matmul(ps[:, :w], ones, ts0[:, :w], start=True, stop=False)
                    nc.tensor.matmul(ps[:, :w], ones, ts1[:, :w], start=False, stop=True)
                    inv = pool_sq.tile([128, CHUNK], F32, tag="inv")
                    scr = pool_sq.tile([128, CHUNK], F32, tag="scr")
                    nc.scalar.activation(scr[:, :w], ps[:, :w], ACTF.Sqrt)
                    # ~18-bit reciprocal: norm factors only need to be
                    # column-coherent; error ~4e-6 << fp32r GEMM noise 1.5e-4
                    nc.vector.reciprocal_approx_fast(inv[:, :w], scr[:, :w])
                    nc.gpsimd.tensor_tensor(
                        t0[:, ck : ck + w], rd(t0[:, ck : ck + w]), inv[:, :w],
                        mybir.AluOpType.mult,
                    )
                    nc.gpsimd.tensor_tensor(
                        t1[:, ck : ck + w], rd(t1[:, ck : ck + w]), inv[:, :w],
                        mybir.AluOpType.mult,
                    )

            # ---- loads all up front (q/ps stream in during GEMM1) ----
            p0, p1 = load_pair(pool_pstr, p_full, ntot, "strp", F32R, pieces=4)
            q0, q1 = load_pair(pool_qstr, q_full, ntot, "strq", BF16, pieces=4)
            qs0, qs1 = load_pair(pool_shard, q_shard, ns, "qs", F32R)
            ps0, ps1 = load_pair(pool_shard, p_shard, ns, "psh", BF16)

            normalize(p0, p1, ntot)
            normalize(qs0, qs1, ns)

            def gemm_scan(w0, w1, s0, s1, vals, idxs, dma_sim):
                """w: [128, ns] weight c-tiles; s: [128, ntot] stream c-tiles.
                Per 128-row group: two half-row blocks, each evicted to SBUF,
                scanned with max8 + find_index8 (8 candidates per half)."""
                for gi in range(g):
                    lhs0 = w0[:, gi * 128 : (gi + 1) * 128]
                    lhs1 = w1[:, gi * 128 : (gi + 1) * 128]
                    for h in range(2):
                        blk = pool_block.tile([128, half], F32, tag="blk")
                        base = h * nch_half
                        cb = 0
                        while cb < nch_half:
                            npair = min(2, nch_half - cb)
                            wdt = npair * CHUNK
                            ps = pool_psum.tile([128, 2 * CHUNK], F32, tag="ps")
                            for lhs, s, st in ((lhs0, s0, True), (lhs1, s1, False)):
                                for i in range(npair):
                                    ck = base + cb + i
                                    nc.tensor.matmul(
                                        ps[:, i * CHUNK : (i + 1) * CHUNK],
                                        lhs,
                                        s[:, ck * CHUNK : (ck + 1) * CHUNK],
                                        start=st, stop=not st,
                                    )
                            nc.scalar.activation(
                                blk[:, cb * CHUNK : cb * CHUNK + wdt],
                                ps[:, :wdt], ACTF.Copy,
                            )
                            cb += npair
                        if dma_sim is not None:
                            nc.sync.dma_start(
                                out=dma_sim[
                                    gi * 128 : (gi + 1) * 128,
                                    h * half : (h + 1) * half,
                                ],
                                in_=blk,
                            )
                        sl = slice(gi * 16 + h * 8, gi * 16 + h * 8 + 8)
                        nc.vector.max(out=vals[:, sl], in_=blk)
                        nc.vector.max_index(
                            out=idxs[:, sl], in_max=vals[:, sl], in_values=blk
                        )

            # ---- GEMM1 (fp32r): sim rows + row-argmax candidates ----
            gemm_scan(qs0, qs1, p0, p1, row_vals, row_idx, sim_out)
            nc.sync.dma_start(out=row_vals_o[:], in_=row_vals)
            nc.sync.dma_start(out=row_idx_o[:], in_=row_idx)

            # ---- q/p_shard norms run in GEMM1's engine tail ----
            normalize(q0, q1, ntot)
            normalize(ps0, ps1, ns)

            # ---- GEMM2 (bf16): col-argmax candidates, no sim writeback ----
            gemm_scan(ps0, ps1, q0, q1, col_vals, col_idx, None)
            nc.sync.dma_start(out=col_vals_o[:], in_=col_vals)
            nc.sync.dma_start(out=col_idx_o[:], in_=col_idx)

    if do_compile:
        nc.compile()
    return nc


_CACHED = {}


def _get_graph():
    if "nc" not in _CACHED:
        _CACHED["nc"] = build_graph()
    return _CACHED["nc"]


def _unpack_cand(arr, g=G, half=NTOT // 2):
    """[128, g*16] -> [g*128, 16] global candidate indices in shard row order.
    Slots 8:16 come from the second half-row block -> +half offset."""
    a = arr.reshape(128, g, 2, 8).astype(np.int64)
    a = a.copy()
    a[:, :, 1, :] += half
    return a.transpose(1, 0, 2, 3).reshape(g * 128, 16)


def _refine_argmax(cand_idx, qn, pn, transpose):
    """Exact f64 rescoring of the device's candidates per row.

    cand_idx: [R, K] candidate indices. For rows (transpose=False) row r of sim
    scores candidates as qn[:, r] . pn[:, cand]; for cols the roles swap.
    Returns (argmax_index [R], max_value [R])."""
    a, b = (qn, pn) if not transpose else (pn, qn)
    scores = np.einsum("cr,crk->rk", a, b[:, cand_idx], optimize=True)
    best = scores.max(axis=1, keepdims=True)
    # break exact ties by smallest index (jnp argmax semantics)
    masked = np.where(scores == best, cand_idx, np.iinfo(np.int64).max)
    idx = masked.min(axis=1)
    return idx, best[:, 0]


def kernel(feature_query: np.ndarray, feature_projection: np.ndarray):
    from concourse.bass_utils import run_bass_kernel_spmd

    q = np.ascontiguousarray(np.asarray(feature_query, np.float32).reshape(C, NTOT))
    p = np.ascontiguousarray(np.asarray(feature_projection, np.float32).reshape(C, NTOT))

    in_maps = []
    for k in range(NCORES):
        sl = slice(k * NS, (k + 1) * NS)
        in_maps.append(
            {
                "q_full": q,
                "p_full": p,
                "q_shard": np.ascontiguousarray(q[:, sl]),
                "p_shard": np.ascontiguousarray(p[:, sl]),
            }
        )

    nc = _get_graph()
    res = run_bass_kernel_spmd(nc, in_maps, core_ids=list(range(NCORES))).results

    sim = np.concatenate([res[k]["sim"] for k in range(NCORES)], axis=0)
    row_cand = np.concatenate([_unpack_cand(res[k]["row_idx"]) for k in range(NCORES)])
    col_cand = np.concatenate([_unpack_cand(res[k]["col_idx"]) for k in range(NCORES)])

    q64 = q.astype(np.float64)
    p64 = p.astype(np.float64)
    qn = q64 / np.linalg.norm(q64, axis=0, keepdims=True)
    pn = p64 / np.linalg.norm(p64, axis=0, keepdims=True)
    row_max, _ = _refine_argmax(row_cand, qn, pn, transpose=False)
    q_idx, sim_vals = _refine_argmax(col_cand, qn, pn, transpose=True)

    j = np.arange(NTOT)
    mutual = (row_max[q_idx] == j) & (sim_vals > THRESHOLD)
    q_div, q_mod = q_idx // W, q_idx % W
    p_div, p_mod = j // W, j % W
    valid = mutual & (q_div + 1 < H) & (p_div + 1 < H)

    i32 = np.int32
    return (
        sim,
        valid,
        q_idx.astype(i32),
        q_div.astype(i32),
        q_mod.astype(i32),
        p_div.astype(i32),
        p_mod.astype(i32),
    )


# revision 28
# speedup vs baseline: 1.2609x; 1.0371x over previous
"""DenseMatcher kernel for 8 TRN2 NeuronCores.

sim = (q/|q|)^T (p/|p|)  [9216, 9216], row-argmax, col-argmax, mutual-NN.

Sharding: core k owns sim rows [k*1152, (k+1)*1152) (GEMM1: q_shard^T @ p_full,
fp32r) and sim cols [k*1152, (k+1)*1152) (GEMM2: p_shard^T @ q_full, bf16,
whose row-argmax is the col-argmax of sim). Device does normalize, both GEMMs
and full top-8 argmax scans per half-row; the host concatenates per-core
results and rescores the (<=16) candidates per row/col at f64 to undo the
reduced-precision GEMM noise (noise ~1e-4..3e-3 << top-2 gap median 1.3e-2,
so the exact argmax is always inside the device candidate set).
"""

import numpy as np
import sys

sys.path.insert(0, "/opt/trn_rl_repo")

import concourse.bacc as bacc
import concourse.bass as bass
import concourse.mybir as mybir
from concourse.tile import TileContext

F32 = mybir.dt.float32
F32R = mybir.dt.float32r
BF16 = mybir.dt.bfloat16
U32 = mybir.dt.uint32

C = 256  # channels (2 partition tiles of 128)
H = 96
W = 96
NTOT = H * W  # 9216
NCORES = 8
NS = NTOT // NCORES  # 1152 rows/cols per core
G = NS // 128  # 9 groups of 128 rows
CHUNK = 512
THRESHOLD = 0.9

ACTF = mybir.ActivationFunctionType


def build_graph(ntot=NTOT, ns=NS, do_compile=True):
    g = ns // 128
    half = ntot // 2
    nch_half = half // CHUNK
    assert half % CHUNK == 0

    nc = bacc.Bacc("TRN2", target_bir_lowering=False, debug=False)
    q_full = nc.declare_dram_parameter("q_full", [C, ntot], F32, isOutput=False)
    p_full = nc.declare_dram_parameter("p_full", [C, ntot], F32, isOutput=False)
    q_shard = nc.declare_dram_parameter("q_shard", [C, ns], F32, isOutput=False)
    p_shard = nc.declare_dram_parameter("p_shard", [C, ns], F32, isOutput=False)

    sim_out = nc.declare_dram_parameter("sim", [ns, ntot], F32, isOutput=True)
    row_vals_o = nc.declare_dram_parameter("row_vals", [128, g * 16], F32, isOutput=True)
    row_idx_o = nc.declare_dram_parameter("row_idx", [128, g * 16], U32, isOutput=True)
    col_vals_o = nc.declare_dram_parameter("col_vals", [128, g * 16], F32, isOutput=True)
    col_idx_o = nc.declare_dram_parameter("col_idx", [128, g * 16], U32, isOutput=True)

    def rd(ap):
        # f32-view for non-matmul readers of fp32r tiles; bf16 reads natively
        return ap.bitcast(F32) if ap.dtype == F32R else ap

    with TileContext(nc) as tc:
        with (
            tc.tile_pool(name="pstr", bufs=1) as pool_pstr,
            tc.tile_pool(name="qstr", bufs=1) as pool_qstr,
            tc.tile_pool(name="block", bufs=2) as pool_block,
            tc.tile_pool(name="shard", bufs=1) as pool_shard,
            tc.tile_pool(name="sq", bufs=2) as pool_sq,
            tc.tile_pool(name="res", bufs=1) as pool_res,
            tc.tile_pool(name="psum", bufs=4, space="PSUM") as pool_psum,
            tc.tile_pool(name="const", bufs=1) as pool_const,
        ):
            ones = pool_const.tile([128, 128], F32, tag="ones")
            nc.vector.memset(ones, 1.0)

            row_vals = pool_res.tile([128, g * 16], F32, tag="rv")
            row_idx = pool_res.tile([128, g * 16], U32, tag="ri")
            col_vals = pool_res.tile([128, g * 16], F32, tag="cv")
            col_idx = pool_res.tile([128, g * 16], U32, tag="ci")

            def load_pair(pool, dram, width, tag, dt, pieces=1):
                # gpsimd casting DMA rounds f32 -> f32r/bf16 on load, so every
                # writer of a matmul input is reduced-precision-rounded.
                # Chunked pieces let the norm pipeline start at the first one.
                t0 = pool.tile([128, width], dt, tag=tag + "0")
                t1 = pool.tile([128, width], dt, tag=tag + "1")
                step = width // pieces
                for o in range(0, width, step):
                    e = min(o + step, width)
                    nc.gpsimd.dma_start(out=t0[:, o:e], in_=dram[0:128, o:e])
                    nc.gpsimd.dma_start(out=t1[:, o:e], in_=dram[128:256, o:e])
                return t0, t1

            def normalize(t0, t1, width):
                """Column-L2-normalize the c-tile pair in place, fully chunked:
                squares (ACT) -> colsum broadcast (PE ones-matmul) -> sqrt (ACT)
                -> approx-reciprocal (DVE) -> scale (Pool). inv chunks live only
                transiently in the sq pool."""
                for ck in range(0, width, CHUNK):
                    w = min(CHUNK, width - ck)
                    ts0 = pool_sq.tile([128, CHUNK], F32, tag="ts0")
                    ts1 = pool_sq.tile([128, CHUNK], F32, tag="ts1")
                    nc.scalar.activation(ts0[:, :w], rd(t0[:, ck : ck + w]), ACTF.Square)
                    nc.scalar.activation(ts1[:, :w], rd(t1[:, ck : ck + w]), ACTF.Square)
                    ps = pool_psum.tile([128, 2 * CHUNK], F32, tag="ps")
                    nc.tensor.matmul(ps[:, :w], ones, ts0[:, :w], start=True, stop=False)
                    nc.tensor.matmul(ps[:, :w], ones, ts1[:, :w], start=False, stop=True)
                    inv = pool_sq.tile([128, CHUNK], F32, tag="inv")
                    scr = pool_sq.tile([128, CHUNK], F32, tag="scr")
                    nc.scalar.activation(scr[:, :w], ps[:, :w], ACTF.Sqrt)
                    # ~18-bit reciprocal: norm factors only need to be
                    # column-coherent; error ~4e-6 << fp32r GEMM noise 1.5e-4
                    nc.vector.reciprocal_approx_fast(inv[:, :w], scr[:, :w])
                    nc.gpsimd.tensor_tensor(
                        t0[:, ck : ck + w], rd(t0[:, ck : ck + w]), inv[:, :w],
                        mybir.AluOpType.mult,
                    )
                    nc.gpsimd.tensor_tensor(
                        t1[:, ck : ck + w], rd(t1[:, ck : ck + w]), inv[:, :w],
                        mybir.AluOpType.mult,
                    )

            # ---- loads all up front (q/ps stream in during GEMM1); the tiny
            # GEMM1-weight shard goes first so its norm clears the head fast
            qs0, qs1 = load_pair(pool_shard, q_shard, ns, "qs", F32R)
            ps0, ps1 = load_pair(pool_shard, p_shard, ns, "psh", BF16)
            p0, p1 = load_pair(pool_pstr, p_full, ntot, "strp", F32R, pieces=8)
            q0, q1 = load_pair(pool_qstr, q_full, ntot, "strq", BF16, pieces=4)

            normalize(qs0, qs1, ns)
            normalize(p0, p1, ntot)

            def gemm_scan(w0, w1, s0, s1, vals, idxs, dma_sim):
                """w: [128, ns] weight c-tiles; s: [128, ntot] stream c-tiles.
                Per 128-row group: two half-row blocks, each evicted to SBUF,
                scanned with max8 + find_index8 (8 candidates per half)."""
                for gi in range(g):
                    lhs0 = w0[:, gi * 128 : (gi + 1) * 128]
                    lhs1 = w1[:, gi * 128 : (gi + 1) * 128]
                    for h in range(2):
                        blk = pool_block.tile([128, half], F32, tag="blk")
                        base = h * nch_half
                        cb = 0
                        while cb < nch_half:
                            npair = min(2, nch_half - cb)
                            wdt = npair * CHUNK
                            ps = pool_psum.tile([128, 2 * CHUNK], F32, tag="ps")
                            for lhs, s, st in ((lhs0, s0, True), (lhs1, s1, False)):
                                for i in range(npair):
                                    ck = base + cb + i
                                    nc.tensor.matmul(
                                        ps[:, i * CHUNK : (i + 1) * CHUNK],
                                        lhs,
                                        s[:, ck * CHUNK : (ck + 1) * CHUNK],
                                        start=st, stop=not st,
                                    )
                            nc.scalar.activation(
                                blk[:, cb * CHUNK : cb * CHUNK + wdt],
                                ps[:, :wdt], ACTF.Copy,
                            )
                            cb += npair
                        if dma_sim is not None:
                            nc.sync.dma_start(
                                out=dma_sim[
                                    gi * 128 : (gi + 1) * 128,
                                    h * half : (h + 1) * half,
                                ],
                                in_=blk,
                            )
                        sl = slice(gi * 16 + h * 8, gi * 16 + h * 8 + 8)
                        nc.vector.max(out=vals[:, sl], in_=blk)
                        nc.vector.max_index(
                            out=idxs[:, sl], in_max=vals[:, sl], in_values=blk
                        )

            # ---- GEMM1 (fp32r): sim rows + row-argmax candidates ----
            gemm_scan(qs0, qs1, p0, p1, row_vals, row_idx, sim_out)
            nc.sync.dma_start(out=row_vals_o[:], in_=row_vals)
            nc.sync.dma_start(out=row_idx_o[:], in_=row_idx)

            # ---- q/p_shard norms run in GEMM1's engine tail ----
            normalize(q0, q1, ntot)
            normalize(ps0, ps1, ns)

            # ---- GEMM2 (bf16): col-argmax candidates, no sim writeback ----
            gemm_scan(ps0, ps1, q0, q1, col_vals, col_idx, None)
            nc.sync.dma_start(out=col_vals_o[:], in_=col_vals)
            nc.sync.dma_start(out=col_idx_o[:], in_=col_idx)

    if do_compile:
        nc.compile()
    return nc


_CACHED = {}


def _get_graph():
    if "nc" not in _CACHED:
        _CACHED["nc"] = build_graph()
    return _CACHED["nc"]


def _unpack_cand(arr, g=G, half=NTOT // 2):
    """[128, g*16] -> [g*128, 16] global candidate indices in shard row order.
    Slots 8:16 come from the second half-row block -> +half offset."""
    a = arr.reshape(128, g, 2, 8).astype(np.int64)
    a = a.copy()
    a[:, :, 1, :] += half
    return a.transpose(1, 0, 2, 3).reshape(g * 128, 16)


def _refine_argmax(cand_idx, qn, pn, transpose):
    """Exact f64 rescoring of the device's candidates per row.

    cand_idx: [R, K] candidate indices. For rows (transpose=False) row r of sim
    scores candidates as qn[:, r] . pn[:, cand]; for cols the roles swap.
    Returns (argmax_index [R], max_value [R])."""
    a, b = (qn, pn) if not transpose else (pn, qn)
    scores = np.einsum("cr,crk->rk", a, b[:, cand_idx], optimize=True)
    best = scores.max(axis=1, keepdims=True)
    # break exact ties by smallest index (jnp argmax semantics)
    masked = np.where(scores == best, cand_idx, np.iinfo(np.int64).max)
    idx = masked.min(axis=1)
    return idx, best[:, 0]


def kernel(feature_query: np.ndarray, feature_projection: np.ndarray):
    from concourse.bass_utils import run_bass_kernel_spmd

    q = np.ascontiguousarray(np.asarray(feature_query, np.float32).reshape(C, NTOT))
    p = np.ascontiguousarray(np.asarray(feature_projection, np.float32).reshape(C, NTOT))

    in_maps = []
    for k in range(NCORES):
        sl = slice(k * NS, (k + 1) * NS)
        in_maps.append(
            {
                "q_full": q,
                "p_full": p,
                "q_shard": np.ascontiguousarray(q[:, sl]),
                "p_shard": np.ascontiguousarray(p[:, sl]),
            }
        )

    nc = _get_graph()
    res = run_bass_kernel_spmd(nc, in_maps, core_ids=list(range(NCORES))).results

    sim = np.concatenate([res[k]["sim"] for k in range(NCORES)], axis=0)
    row_cand = np.concatenate([_unpack_cand(res[k]["row_idx"]) for k in range(NCORES)])
    col_cand = np.concatenate([_unpack_cand(res[k]["col_idx"]) for k in range(NCORES)])

    q64 = q.astype(np.float64)
    p64 = p.astype(np.float64)
    qn = q64 / np.linalg.norm(q64, axis=0, keepdims=True)
    pn = p64 / np.linalg.norm(p64, axis=0, keepdims=True)
    row_max, _ = _refine_argmax(row_cand, qn, pn, transpose=False)
    q_idx, sim_vals = _refine_argmax(col_cand, qn, pn, transpose=True)

    j = np.arange(NTOT)
    mutual = (row_max[q_idx] == j) & (sim_vals > THRESHOLD)
    q_div, q_mod = q_idx // W, q_idx % W
    p_div, p_mod = j // W, j % W
    valid = mutual & (q_div + 1 < H) & (p_div + 1 < H)

    i32 = np.int32
    return (
        sim,
        valid,
        q_idx.astype(i32),
        q_div.astype(i32),
        q_mod.astype(i32),
        p_div.astype(i32),
        p_mod.astype(i32),
    )


# revision 29
# speedup vs baseline: 1.2630x; 1.0017x over previous
"""DenseMatcher kernel for 8 TRN2 NeuronCores.

sim = (q/|q|)^T (p/|p|)  [9216, 9216], row-argmax, col-argmax, mutual-NN.

Sharding: core k owns sim rows [k*1152, (k+1)*1152) (GEMM1: q_shard^T @ p_full,
fp32r) and sim cols [k*1152, (k+1)*1152) (GEMM2: p_shard^T @ q_full, bf16,
whose row-argmax is the col-argmax of sim). Device does normalize, both GEMMs
and full top-8 argmax scans per half-row; the host concatenates per-core
results and rescores the (<=16) candidates per row/col at f64 to undo the
reduced-precision GEMM noise (noise ~1e-4..3e-3 << top-2 gap median 1.3e-2,
so the exact argmax is always inside the device candidate set).
"""

import numpy as np
import sys

sys.path.insert(0, "/opt/trn_rl_repo")

import concourse.bacc as bacc
import concourse.bass as bass
import concourse.mybir as mybir
from concourse.tile import TileContext

F32 = mybir.dt.float32
F32R = mybir.dt.float32r
BF16 = mybir.dt.bfloat16
U32 = mybir.dt.uint32

C = 256  # channels (2 partition tiles of 128)
H = 96
W = 96
NTOT = H * W  # 9216
NCORES = 8
NS = NTOT // NCORES  # 1152 rows/cols per core
G = NS // 128  # 9 groups of 128 rows
CHUNK = 512
THRESHOLD = 0.9

ACTF = mybir.ActivationFunctionType


def build_graph(ntot=NTOT, ns=NS, do_compile=True):
    g = ns // 128
    half = ntot // 2
    nch_half = half // CHUNK
    assert half % CHUNK == 0

    nc = bacc.Bacc("TRN2", target_bir_lowering=False, debug=False)
    q_full = nc.declare_dram_parameter("q_full", [C, ntot], F32, isOutput=False)
    p_full = nc.declare_dram_parameter("p_full", [C, ntot], F32, isOutput=False)
    q_shard = nc.declare_dram_parameter("q_shard", [C, ns], F32, isOutput=False)
    p_shard = nc.declare_dram_parameter("p_shard", [C, ns], F32, isOutput=False)

    sim_out = nc.declare_dram_parameter("sim", [ns, ntot], F32, isOutput=True)
    row_vals_o = nc.declare_dram_parameter("row_vals", [128, g * 16], F32, isOutput=True)
    row_idx_o = nc.declare_dram_parameter("row_idx", [128, g * 16], U32, isOutput=True)
    col_vals_o = nc.declare_dram_parameter("col_vals", [128, g * 16], F32, isOutput=True)
    col_idx_o = nc.declare_dram_parameter("col_idx", [128, g * 16], U32, isOutput=True)

    def rd(ap):
        # f32-view for non-matmul readers of fp32r tiles; bf16 reads natively
        return ap.bitcast(F32) if ap.dtype == F32R else ap

    with TileContext(nc) as tc:
        with (
            tc.tile_pool(name="pstr", bufs=1) as pool_pstr,
            tc.tile_pool(name="qstr", bufs=1) as pool_qstr,
            tc.tile_pool(name="block", bufs=3) as pool_block,
            tc.tile_pool(name="shard", bufs=1) as pool_shard,
            tc.tile_pool(name="sq", bufs=2) as pool_sq,
            tc.tile_pool(name="res", bufs=1) as pool_res,
            tc.tile_pool(name="psum", bufs=4, space="PSUM") as pool_psum,
            tc.tile_pool(name="const", bufs=1) as pool_const,
        ):
            ones = pool_const.tile([128, 128], F32, tag="ones")
            nc.vector.memset(ones, 1.0)

            row_vals = pool_res.tile([128, g * 16], F32, tag="rv")
            row_idx = pool_res.tile([128, g * 16], U32, tag="ri")
            col_vals = pool_res.tile([128, g * 16], F32, tag="cv")
            col_idx = pool_res.tile([128, g * 16], U32, tag="ci")

            def load_pair(pool, dram, width, tag, dt, pieces=1):
                # gpsimd casting DMA rounds f32 -> f32r/bf16 on load, so every
                # writer of a matmul input is reduced-precision-rounded.
                # Chunked pieces let the norm pipeline start at the first one.
                t0 = pool.tile([128, width], dt, tag=tag + "0")
                t1 = pool.tile([128, width], dt, tag=tag + "1")
                step = width // pieces
                for o in range(0, width, step):
                    e = min(o + step, width)
                    nc.gpsimd.dma_start(out=t0[:, o:e], in_=dram[0:128, o:e])
                    nc.gpsimd.dma_start(out=t1[:, o:e], in_=dram[128:256, o:e])
                return t0, t1

            def normalize(t0, t1, width):
                """Column-L2-normalize the c-tile pair in place, fully chunked:
                squares (ACT) -> colsum broadcast (PE ones-matmul) -> sqrt (ACT)
                -> approx-reciprocal (DVE) -> scale (Pool). inv chunks live only
                transiently in the sq pool."""
                for ck in range(0, width, CHUNK):
                    w = min(CHUNK, width - ck)
                    ts0 = pool_sq.tile([128, CHUNK], F32, tag="ts0")
                    ts1 = pool_sq.tile([128, CHUNK], F32, tag="ts1")
                    nc.scalar.activation(ts0[:, :w], rd(t0[:, ck : ck + w]), ACTF.Square)
                    nc.scalar.activation(ts1[:, :w], rd(t1[:, ck : ck + w]), ACTF.Square)
                    ps = pool_psum.tile([128, 2 * CHUNK], F32, tag="ps")
                    nc.tensor.matmul(ps[:, :w], ones, ts0[:, :w], start=True, stop=False)
                    nc.tensor.matmul(ps[:, :w], ones, ts1[:, :w], start=False, stop=True)
                    inv = pool_sq.tile([128, CHUNK], F32, tag="inv")
                    scr = pool_sq.tile([128, CHUNK], F32, tag="scr")
                    nc.scalar.activation(scr[:, :w], ps[:, :w], ACTF.Sqrt)
                    # ~18-bit reciprocal: norm factors only need to be
                    # column-coherent; error ~4e-6 << fp32r GEMM noise 1.5e-4
                    nc.vector.reciprocal_approx_fast(inv[:, :w], scr[:, :w])
                    nc.gpsimd.tensor_tensor(
                        t0[:, ck : ck + w], rd(t0[:, ck : ck + w]), inv[:, :w],
                        mybir.AluOpType.mult,
                    )
                    nc.gpsimd.tensor_tensor(
                        t1[:, ck : ck + w], rd(t1[:, ck : ck + w]), inv[:, :w],
                        mybir.AluOpType.mult,
                    )

            # ---- loads all up front (q/ps stream in during GEMM1); the tiny
            # GEMM1-weight shard goes first so its norm clears the head fast
            qs0, qs1 = load_pair(pool_shard, q_shard, ns, "qs", F32R)
            ps0, ps1 = load_pair(pool_shard, p_shard, ns, "psh", BF16)
            p0, p1 = load_pair(pool_pstr, p_full, ntot, "strp", F32R, pieces=8)
            q0, q1 = load_pair(pool_qstr, q_full, ntot, "strq", BF16, pieces=4)

            normalize(qs0, qs1, ns)
            normalize(p0, p1, ntot)

            def gemm_scan(w0, w1, s0, s1, vals, idxs, dma_sim):
                """w: [128, ns] weight c-tiles; s: [128, ntot] stream c-tiles.
                Per 128-row group: two half-row blocks, each evicted to SBUF,
                scanned with max8 + find_index8 (8 candidates per half)."""
                for gi in range(g):
                    lhs0 = w0[:, gi * 128 : (gi + 1) * 128]
                    lhs1 = w1[:, gi * 128 : (gi + 1) * 128]
                    for h in range(2):
                        blk = pool_block.tile([128, half], F32, tag="blk")
                        base = h * nch_half
                        cb = 0
                        while cb < nch_half:
                            npair = min(2, nch_half - cb)
                            wdt = npair * CHUNK
                            ps = pool_psum.tile([128, 2 * CHUNK], F32, tag="ps")
                            for lhs, s, st in ((lhs0, s0, True), (lhs1, s1, False)):
                                for i in range(npair):
                                    ck = base + cb + i
                                    nc.tensor.matmul(
                                        ps[:, i * CHUNK : (i + 1) * CHUNK],
                                        lhs,
                                        s[:, ck * CHUNK : (ck + 1) * CHUNK],
                                        start=st, stop=not st,
                                    )
                            nc.scalar.activation(
                                blk[:, cb * CHUNK : cb * CHUNK + wdt],
                                ps[:, :wdt], ACTF.Copy,
                            )
                            cb += npair
                        if dma_sim is not None:
                            nc.sync.dma_start(
                                out=dma_sim[
                                    gi * 128 : (gi + 1) * 128,
                                    h * half : (h + 1) * half,
                                ],
                                in_=blk,
                            )
                        sl = slice(gi * 16 + h * 8, gi * 16 + h * 8 + 8)
                        nc.vector.max(out=vals[:, sl], in_=blk)
                        nc.vector.max_index(
                            out=idxs[:, sl], in_max=vals[:, sl], in_values=blk
                        )

            # ---- GEMM1 (fp32r): sim rows + row-argmax candidates ----
            gemm_scan(qs0, qs1, p0, p1, row_vals, row_idx, sim_out)
            nc.sync.dma_start(out=row_vals_o[:], in_=row_vals)
            nc.sync.dma_start(out=row_idx_o[:], in_=row_idx)

            # ---- q/p_shard norms run in GEMM1's engine tail ----
            normalize(q0, q1, ntot)
            normalize(ps0, ps1, ns)

            # ---- GEMM2 (bf16): col-argmax candidates, no sim writeback ----
            gemm_scan(ps0, ps1, q0, q1, col_vals, col_idx, None)
            nc.sync.dma_start(out=col_vals_o[:], in_=col_vals)
            nc.sync.dma_start(out=col_idx_o[:], in_=col_idx)

    if do_compile:
        nc.compile()
    return nc


_CACHED = {}


def _get_graph():
    if "nc" not in _CACHED:
        _CACHED["nc"] = build_graph()
    return _CACHED["nc"]


def _unpack_cand(arr, g=G, half=NTOT // 2):
    """[128, g*16] -> [g*128, 16] global candidate indices in shard row order.
    Slots 8:16 come from the second half-row block -> +half offset."""
    a = arr.reshape(128, g, 2, 8).astype(np.int64)
    a = a.copy()
    a[:, :, 1, :] += half
    return a.transpose(1, 0, 2, 3).reshape(g * 128, 16)


def _refine_argmax(cand_idx, qn, pn, transpose):
    """Exact f64 rescoring of the device's candidates per row.

    cand_idx: [R, K] candidate indices. For rows (transpose=False) row r of sim
    scores candidates as qn[:, r] . pn[:, cand]; for cols the roles swap.
    Returns (argmax_index [R], max_value [R])."""
    a, b = (qn, pn) if not transpose else (pn, qn)
    scores = np.einsum("cr,crk->rk", a, b[:, cand_idx], optimize=True)
    best = scores.max(axis=1, keepdims=True)
    # break exact ties by smallest index (jnp argmax semantics)
    masked = np.where(scores == best, cand_idx, np.iinfo(np.int64).max)
    idx = masked.min(axis=1)
    return idx, best[:, 0]


def kernel(feature_query: np.ndarray, feature_projection: np.ndarray):
    from concourse.bass_utils import run_bass_kernel_spmd

    q = np.ascontiguousarray(np.asarray(feature_query, np.float32).reshape(C, NTOT))
    p = np.ascontiguousarray(np.asarray(feature_projection, np.float32).reshape(C, NTOT))

    in_maps = []
    for k in range(NCORES):
        sl = slice(k * NS, (k + 1) * NS)
        in_maps.append(
            {
                "q_full": q,
                "p_full": p,
                "q_shard": np.ascontiguousarray(q[:, sl]),
                "p_shard": np.ascontiguousarray(p[:, sl]),
            }
        )

    nc = _get_graph()
    res = run_bass_kernel_spmd(nc, in_maps, core_ids=list(range(NCORES))).results

    sim = np.concatenate([res[k]["sim"] for k in range(NCORES)], axis=0)
    row_cand = np.concatenate([_unpack_cand(res[k]["row_idx"]) for k in range(NCORES)])
    col_cand = np.concatenate([_unpack_cand(res[k]["col_idx"]) for k in range(NCORES)])

    q64 = q.astype(np.float64)
    p64 = p.astype(np.float64)
    qn = q64 / np.linalg.norm(q64, axis=0, keepdims=True)
    pn = p64 / np.linalg.norm(p64, axis=0, keepdims=True)
    row_max, _ = _refine_argmax(row_cand, qn, pn, transpose=False)
    q_idx, sim_vals = _refine_argmax(col_cand, qn, pn, transpose=True)

    j = np.arange(NTOT)
    mutual = (row_max[q_idx] == j) & (sim_vals > THRESHOLD)
    q_div, q_mod = q_idx // W, q_idx % W
    p_div, p_mod = j // W, j % W
    valid = mutual & (q_div + 1 < H) & (p_div + 1 < H)

    i32 = np.int32
    return (
        sim,
        valid,
        q_idx.astype(i32),
        q_div.astype(i32),
        q_mod.astype(i32),
        p_div.astype(i32),
        p_mod.astype(i32),
    )


# revision 32
# speedup vs baseline: 1.2757x; 1.0100x over previous
"""DenseMatcher kernel for 8 TRN2 NeuronCores.

sim = (q/|q|)^T (p/|p|)  [9216, 9216], row-argmax, col-argmax, mutual-NN.

Sharding: core k owns sim rows [k*1152, (k+1)*1152) (GEMM1: q_shard^T @ p_full,
fp32r) and sim cols [k*1152, (k+1)*1152) (GEMM2: p_shard^T @ q_full, bf16,
whose row-argmax is the col-argmax of sim). Device does normalize, both GEMMs
and full top-8 argmax scans per half-row; the host concatenates per-core
results and rescores the (<=16) candidates per row/col at f64 to undo the
reduced-precision GEMM noise (noise ~1e-4..3e-3 << top-2 gap median 1.3e-2,
so the exact argmax is always inside the device candidate set).
"""

import numpy as np
import sys

sys.path.insert(0, "/opt/trn_rl_repo")

import concourse.bacc as bacc
import concourse.bass as bass
import concourse.mybir as mybir
from concourse.tile import TileContext

F32 = mybir.dt.float32
F32R = mybir.dt.float32r
BF16 = mybir.dt.bfloat16
U32 = mybir.dt.uint32

C = 256  # channels (2 partition tiles of 128)
H = 96
W = 96
NTOT = H * W  # 9216
NCORES = 8
NS = NTOT // NCORES  # 1152 rows/cols per core
G = NS // 128  # 9 groups of 128 rows
CHUNK = 512
THRESHOLD = 0.9

ACTF = mybir.ActivationFunctionType


def build_graph(ntot=NTOT, ns=NS, do_compile=True):
    g = ns // 128
    half = ntot // 2
    nch_half = half // CHUNK
    assert half % CHUNK == 0

    nc = bacc.Bacc("TRN2", target_bir_lowering=False, debug=False)
    q_full = nc.declare_dram_parameter("q_full", [C, ntot], F32, isOutput=False)
    p_full = nc.declare_dram_parameter("p_full", [C, ntot], F32, isOutput=False)
    q_shard = nc.declare_dram_parameter("q_shard", [C, ns], F32, isOutput=False)
    p_shard = nc.declare_dram_parameter("p_shard", [C, ns], F32, isOutput=False)

    sim_out = nc.declare_dram_parameter("sim", [ns, ntot], F32, isOutput=True)
    row_vals_o = nc.declare_dram_parameter("row_vals", [128, g * 16], F32, isOutput=True)
    row_idx_o = nc.declare_dram_parameter("row_idx", [128, g * 16], U32, isOutput=True)
    col_vals_o = nc.declare_dram_parameter("col_vals", [128, g * 16], F32, isOutput=True)
    col_idx_o = nc.declare_dram_parameter("col_idx", [128, g * 16], U32, isOutput=True)

    def rd(ap):
        # f32-view for non-matmul readers of fp32r tiles; bf16 reads natively
        return ap.bitcast(F32) if ap.dtype == F32R else ap

    with TileContext(nc) as tc:
        with (
            tc.tile_pool(name="pstr", bufs=1) as pool_pstr,
            tc.tile_pool(name="qstr", bufs=1) as pool_qstr,
            tc.tile_pool(name="block", bufs=3) as pool_block,
            tc.tile_pool(name="shard", bufs=1) as pool_shard,
            tc.tile_pool(name="sq", bufs=3) as pool_sq,
            tc.tile_pool(name="res", bufs=1) as pool_res,
            tc.tile_pool(name="psum", bufs=4, space="PSUM") as pool_psum,
            tc.tile_pool(name="const", bufs=1) as pool_const,
        ):
            ones = pool_const.tile([128, 128], F32, tag="ones")
            nc.vector.memset(ones, 1.0)

            row_vals = pool_res.tile([128, g * 16], F32, tag="rv")
            row_idx = pool_res.tile([128, g * 16], U32, tag="ri")
            col_vals = pool_res.tile([128, g * 16], F32, tag="cv")
            col_idx = pool_res.tile([128, g * 16], U32, tag="ci")

            def load_pair(pool, dram, width, tag, dt, pieces=1):
                # gpsimd casting DMA rounds f32 -> f32r/bf16 on load, so every
                # writer of a matmul input is reduced-precision-rounded.
                # Chunked pieces let the norm pipeline start at the first one.
                t0 = pool.tile([128, width], dt, tag=tag + "0")
                t1 = pool.tile([128, width], dt, tag=tag + "1")
                step = width // pieces
                for o in range(0, width, step):
                    e = min(o + step, width)
                    nc.gpsimd.dma_start(out=t0[:, o:e], in_=dram[0:128, o:e])
                    nc.gpsimd.dma_start(out=t1[:, o:e], in_=dram[128:256, o:e])
                return t0, t1

            def normalize(t0, t1, width):
                """Column-L2-normalize the c-tile pair in place, fully chunked:
                squares (ACT) -> colsum broadcast (PE ones-matmul) -> sqrt (ACT)
                -> approx-reciprocal (DVE) -> scale (Pool). inv chunks live only
                transiently in the sq pool."""
                for ck in range(0, width, CHUNK):
                    w = min(CHUNK, width - ck)
                    ts0 = pool_sq.tile([128, CHUNK], F32, tag="ts0")
                    ts1 = pool_sq.tile([128, CHUNK], F32, tag="ts1")
                    nc.scalar.activation(ts0[:, :w], rd(t0[:, ck : ck + w]), ACTF.Square)
                    nc.scalar.activation(ts1[:, :w], rd(t1[:, ck : ck + w]), ACTF.Square)
                    ps = pool_psum.tile([128, 2 * CHUNK], F32, tag="ps")
                    nc.tensor.matmul(ps[:, :w], ones, ts0[:, :w], start=True, stop=False)
                    nc.tensor.matmul(ps[:, :w], ones, ts1[:, :w], start=False, stop=True)
                    inv = pool_sq.tile([128, CHUNK], F32, tag="inv")
                    scr = pool_sq.tile([128, CHUNK], F32, tag="scr")
                    nc.scalar.activation(scr[:, :w], ps[:, :w], ACTF.Sqrt)
                    # ~18-bit reciprocal: norm factors only need to be
                    # column-coherent; error ~4e-6 << fp32r GEMM noise 1.5e-4
                    nc.vector.reciprocal_approx_fast(inv[:, :w], scr[:, :w])
                    nc.gpsimd.tensor_tensor(
                        t0[:, ck : ck + w], rd(t0[:, ck : ck + w]), inv[:, :w],
                        mybir.AluOpType.mult,
                    )
                    nc.gpsimd.tensor_tensor(
                        t1[:, ck : ck + w], rd(t1[:, ck : ck + w]), inv[:, :w],
                        mybir.AluOpType.mult,
                    )

            # ---- loads all up front (q/ps stream in during GEMM1); the tiny
            # GEMM1-weight shard goes first so its norm clears the head fast
            qs0, qs1 = load_pair(pool_shard, q_shard, ns, "qs", F32R)
            ps0, ps1 = load_pair(pool_shard, p_shard, ns, "psh", BF16)
            p0, p1 = load_pair(pool_pstr, p_full, ntot, "strp", F32R, pieces=8)
            q0, q1 = load_pair(pool_qstr, q_full, ntot, "strq", BF16, pieces=4)

            normalize(qs0, qs1, ns)
            normalize(p0, p1, ntot)
            # GEMM2's inputs normalize up front too: their DVE recips fill the
            # idle head and Pool scales q during GEMM1, so GEMM2 starts with
            # no normalization stall after GEMM1's last scan.
            normalize(q0, q1, ntot)
            normalize(ps0, ps1, ns)

            def gemm_scan(w0, w1, s0, s1, vals, idxs, dma_sim):
                """w: [128, ns] weight c-tiles; s: [128, ntot] stream c-tiles.
                Per 128-row group: two half-row blocks, each evicted to SBUF,
                scanned with max8 + find_index8 (8 candidates per half)."""
                for gi in range(g):
                    lhs0 = w0[:, gi * 128 : (gi + 1) * 128]
                    lhs1 = w1[:, gi * 128 : (gi + 1) * 128]
                    for h in range(2):
                        blk = pool_block.tile([128, half], F32, tag="blk")
                        base = h * nch_half
                        cb = 0
                        while cb < nch_half:
                            npair = min(2, nch_half - cb)
                            wdt = npair * CHUNK
                            ps = pool_psum.tile([128, 2 * CHUNK], F32, tag="ps")
                            for lhs, s, st in ((lhs0, s0, True), (lhs1, s1, False)):
                                for i in range(npair):
                                    ck = base + cb + i
                                    nc.tensor.matmul(
                                        ps[:, i * CHUNK : (i + 1) * CHUNK],
                                        lhs,
                                        s[:, ck * CHUNK : (ck + 1) * CHUNK],
                                        start=st, stop=not st,
                                    )
                            nc.scalar.activation(
                                blk[:, cb * CHUNK : cb * CHUNK + wdt],
                                ps[:, :wdt], ACTF.Copy,
                            )
                            cb += npair
                        if dma_sim is not None:
                            nc.sync.dma_start(
                                out=dma_sim[
                                    gi * 128 : (gi + 1) * 128,
                                    h * half : (h + 1) * half,
                                ],
                                in_=blk,
                            )
                        sl = slice(gi * 16 + h * 8, gi * 16 + h * 8 + 8)
                        nc.vector.max(out=vals[:, sl], in_=blk)
                        nc.vector.max_index(
                            out=idxs[:, sl], in_max=vals[:, sl], in_values=blk
                        )

            # ---- GEMM1 (fp32r): sim rows + row-argmax candidates ----
            gemm_scan(qs0, qs1, p0, p1, row_vals, row_idx, sim_out)
            nc.sync.dma_start(out=row_vals_o[:], in_=row_vals)
            nc.sync.dma_start(out=row_idx_o[:], in_=row_idx)

            # ---- GEMM2 (bf16): col-argmax candidates, no sim writeback ----
            gemm_scan(ps0, ps1, q0, q1, col_vals, col_idx, None)
            nc.sync.dma_start(out=col_vals_o[:], in_=col_vals)
            nc.sync.dma_start(out=col_idx_o[:], in_=col_idx)

    if do_compile:
        nc.compile()
    return nc


_CACHED = {}


def _get_graph():
    if "nc" not in _CACHED:
        _CACHED["nc"] = build_graph()
    return _CACHED["nc"]


def _unpack_cand(arr, g=G, half=NTOT // 2):
    """[128, g*16] -> [g*128, 16] global candidate indices in shard row order.
    Slots 8:16 come from the second half-row block -> +half offset."""
    a = arr.reshape(128, g, 2, 8).astype(np.int64)
    a = a.copy()
    a[:, :, 1, :] += half
    return a.transpose(1, 0, 2, 3).reshape(g * 128, 16)


def _refine_argmax(cand_idx, qn, pn, transpose):
    """Exact f64 rescoring of the device's candidates per row.

    cand_idx: [R, K] candidate indices. For rows (transpose=False) row r of sim
    scores candidates as qn[:, r] . pn[:, cand]; for cols the roles swap.
    Returns (argmax_index [R], max_value [R])."""
    a, b = (qn, pn) if not transpose else (pn, qn)
    scores = np.einsum("cr,crk->rk", a, b[:, cand_idx], optimize=True)
    best = scores.max(axis=1, keepdims=True)
    # break exact ties by smallest index (jnp argmax semantics)
    masked = np.where(scores == best, cand_idx, np.iinfo(np.int64).max)
    idx = masked.min(axis=1)
    return idx, best[:, 0]


def kernel(feature_query: np.ndarray, feature_projection: np.ndarray):
    from concourse.bass_utils import run_bass_kernel_spmd

    q = np.ascontiguousarray(np.asarray(feature_query, np.float32).reshape(C, NTOT))
    p = np.ascontiguousarray(np.asarray(feature_projection, np.float32).reshape(C, NTOT))

    in_maps = []
    for k in range(NCORES):
        sl = slice(k * NS, (k + 1) * NS)
        in_maps.append(
            {
                "q_full": q,
                "p_full": p,
                "q_shard": np.ascontiguousarray(q[:, sl]),
                "p_shard": np.ascontiguousarray(p[:, sl]),
            }
        )

    nc = _get_graph()
    res = run_bass_kernel_spmd(nc, in_maps, core_ids=list(range(NCORES))).results

    sim = np.concatenate([res[k]["sim"] for k in range(NCORES)], axis=0)
    row_cand = np.concatenate([_unpack_cand(res[k]["row_idx"]) for k in range(NCORES)])
    col_cand = np.concatenate([_unpack_cand(res[k]["col_idx"]) for k in range(NCORES)])

    q64 = q.astype(np.float64)
    p64 = p.astype(np.float64)
    qn = q64 / np.linalg.norm(q64, axis=0, keepdims=True)
    pn = p64 / np.linalg.norm(p64, axis=0, keepdims=True)
    row_max, _ = _refine_argmax(row_cand, qn, pn, transpose=False)
    q_idx, sim_vals = _refine_argmax(col_cand, qn, pn, transpose=True)

    j = np.arange(NTOT)
    mutual = (row_max[q_idx] == j) & (sim_vals > THRESHOLD)
    q_div, q_mod = q_idx // W, q_idx % W
    p_div, p_mod = j // W, j % W
    valid = mutual & (q_div + 1 < H) & (p_div + 1 < H)

    i32 = np.int32
    return (
        sim,
        valid,
        q_idx.astype(i32),
        q_div.astype(i32),
        q_mod.astype(i32),
        p_div.astype(i32),
        p_mod.astype(i32),
    )
